# revision 53
# baseline (speedup 1.0000x reference)
"""DynamicProxyNCA loss on 8 TRN2 NeuronCores (Bass/Tile, SPMD).

Class-block decomposition: the hardest-positive argmax for an anchor only
ranges over same-class columns in its suffix (~132 of 8192), so scores are
computed only within class blocks instead of over the full batch.

  - Host (index prep only): permute columns by class; FFD-pack classes into
    bins of <=128 anchors x <=W columns; bin b -> core b%8, group b//8.
    Suffix+class+padding constraints become one additive bf16 mask per bin.
  - Device, per group g: nearest-proxy selection for the group's anchors
    (E2 = 2<z_a,p> - w, max8/find_index), then ONE f32r matmul pair builds
    scores S = -2<p_sel, z_j> + zz_j directly in PSUM (zz via an all-ones
    lhsT over squared columns), + mask add -> strip -> max8/find_index.
  - Hardest positive gathered by indirect DMA from the permuted row table
    (f32, exact); D_p / D_n / logsumexp recomputed in f32 as in reference.
  - All sqrt as exp(0.5*ln(x)): the whole program runs on ONE scalar
    activation table (natural_log_exp covers Square/Copy/Exp/Ln).
  - ||prx_n||^2 == 1 by construction, so bb is the constant 1.0.
  - Host sums per-anchor losses (pure gather/reduce of per-core outputs).
"""
import sys

sys.path.insert(0, "/opt/trn_rl_repo")

import numpy as np
import ml_dtypes

import concourse.bass as bass
import concourse.tile as tile
from concourse import bacc, mybir
from concourse.bass_utils import run_bass_kernel_spmd
from concourse.masks import make_identity

# This kernel only uses Square/Copy/Exp/Ln on the scalar engine, all of which
# live together in the natural_log_exp_and_others activation table. The
# default table-selection pass resolves each function to the FIRST table
# containing it, which thrashes between exp_and_others and natural_log
# (1.5us reload each). Restrict resolution to the one covering table while
# preserving table ids (dict order); runtime behavior is identical since that
# table genuinely contains every function used.
from concourse.hw_specs import get_activation_tables as _real_gat

_COVER_SET = "natural_log_exp_and_others"


def _gat_single(arch):
    tabs = _real_gat(arch)
    return {n: (f if n == _COVER_SET else set()) for n, f in tabs.items()}


bacc.get_activation_tables = _gat_single

F32 = mybir.dt.float32
F32R = mybir.dt.float32r
BF16 = mybir.dt.bfloat16
U32 = mybir.dt.uint32

B, Z = 8192, 128
P = 93
EPS = 1e-6
RT = 128                # max anchors per group
NCORE = 8
BIGNEG = -9984.0        # exact in bf16; far below any real score (>= -1)
EPS2 = 2.0 * EPS
ZEPS2 = Z * EPS * EPS

_CACHE = {}


def build_program(G, W):
    nc = bacc.Bacc(None, target_bir_lowering=False, debug=False)

    ztl = nc.dram_tensor("ztl", [Z, G * W], F32R, kind="ExternalInput")
    zatl = nc.dram_tensor("zatl", [Z, G * RT], F32, kind="ExternalInput")
    maskb = nc.dram_tensor("maskb", [RT, G * W], BF16, kind="ExternalInput")
    zrlp = nc.dram_tensor("zrlp", [G * W, Z], F32, kind="ExternalInput")
    prx_in = nc.dram_tensor("prx", [P, Z], F32, kind="ExternalInput")
    iota93_in = nc.dram_tensor("iota93", [RT, P], F32, kind="ExternalInput")
    out = nc.dram_tensor("out", [RT, 2 * G], F32, kind="ExternalOutput")

    AL = mybir.AluOpType
    AF = mybir.ActivationFunctionType
    AX = mybir.AxisListType

    from contextlib import ExitStack

    with tile.TileContext(nc) as tc, ExitStack() as ctx:
        singles = ctx.enter_context(tc.tile_pool(name="singles", bufs=1))

        prx = singles.tile([P, Z], F32)
        nc.scalar.dma_start(out=prx[:, :], in_=prx_in[:, :])
        zat = singles.tile([Z, G * RT], F32)
        nc.sync.dma_start(out=zat[:, :], in_=zatl[:, :])
        iota93 = singles.tile([RT, P], F32)
        nc.scalar.dma_start(out=iota93[:, :], in_=iota93_in[:, :])
        zTr = singles.tile([Z, G * W], F32R)
        nc.sync.dma_start(out=zTr[:, :], in_=ztl[:, :])
        maskT = singles.tile([RT, G * W], BF16)
        nc.scalar.dma_start(out=maskT[:, :], in_=maskb[:, :])

        identity = singles.tile([128, 128], F32)
        make_identity(nc, identity[:, :])
        onescol = singles.tile([1, RT], F32)
        nc.vector.memset(onescol[:, :], 1.0)
        ones128 = singles.tile([Z, RT], F32)
        nc.vector.memset(ones128[:, :], 1.0)
        outbuf = singles.tile([RT, 2 * G], F32)

        # ---- proxy preprocessing (sqrt-free: rn = exp(-0.5*ln(ss)))
        with tc.tile_pool(name="setup_sb", bufs=1) as stp, \
             tc.tile_pool(name="setup_ps", bufs=1, space="PSUM") as stps:
            scratch = stp.tile([P, Z], F32)
            ss = stp.tile([P, 1], F32)
            nc.scalar.activation(out=scratch[:, :], in_=prx[:, :], func=AF.Square,
                                 accum_out=ss[:, :])
            lnss = stp.tile([P, 1], F32)
            nc.scalar.activation(out=lnss[:, :], in_=ss[:, :], func=AF.Ln)
            rn = stp.tile([P, 1], F32)
            nc.scalar.activation(out=rn[:, :], in_=lnss[:, :], func=AF.Exp,
                                 scale=-0.5)
            prx_n = stp.tile([P, Z], F32)
            nc.vector.tensor_scalar_mul(out=prx_n[:, :], in0=prx[:, :], scalar1=rn[:, :])
            m2prx = singles.tile([P, Z], F32)
            nc.vector.tensor_scalar_mul(out=m2prx[:, :], in0=prx_n[:, :], scalar1=-2.0)
            sb0 = stp.tile([P, 1], F32)
            nc.vector.tensor_reduce(out=sb0[:, :], in_=prx_n[:, :], axis=AX.X, op=AL.add)

            ps_t = stps.tile([Z, P], F32, tag="pst")
            nc.tensor.transpose(out=ps_t[:, :], in_=prx_n[:, :], identity=identity[:P, :P])
            prxT2 = singles.tile([Z, P], F32)
            nc.scalar.mul(out=prxT2[:, :], in_=ps_t[:, :], mul=2.0)
            mprxT = singles.tile([Z, P], F32)
            nc.scalar.mul(out=mprxT[:, :], in_=ps_t[:, :], mul=-2.0)

            ps_r2 = stps.tile([1, P], F32, tag="psr")
            nc.tensor.transpose(out=ps_r2[:, :], in_=sb0[:, :], identity=identity[:P, :P])
            sbrow = singles.tile([1, P], F32)
            nc.vector.tensor_copy(out=sbrow[:, :], in_=ps_r2[:, :])
            # bb == ||prx_n||^2 == 1 by construction: w = 1 - 2*eps*sb
            wrow = singles.tile([1, P], F32)
            nc.vector.tensor_scalar(out=wrow[:, :], in0=sbrow[:, :], scalar1=-EPS2,
                                    scalar2=1.0, op0=AL.mult, op1=AL.add)

        gsb0 = ctx.enter_context(tc.tile_pool(name="gsb0", bufs=1))

        gsb = ctx.enter_context(tc.tile_pool(name="gsb", bufs=1))
        ps = ctx.enter_context(tc.tile_pool(name="ps", bufs=2, space="PSUM"))
        ps_s = ctx.enter_context(tc.tile_pool(name="ps_s", bufs=2, space="PSUM"))
        ps_oh = ctx.enter_context(tc.tile_pool(name="ps_oh", bufs=1, space="PSUM"))

        # ---- prelim stage A: nearest proxy per anchor (argmax of E2)
        onehots = []
        for g in range(G):
            a0 = g * RT
            ps_e2 = ps.tile([RT, P], F32, tag="EP")
            nc.tensor.matmul(ps_e2[:, :], lhsT=zat[:, a0:a0 + RT], rhs=prxT2[:, :],
                             start=True, stop=True)
            # argmin tie-term -2eps*sb_k (~2e-5) is 50x below f32r matmul
            # noise; dropping it lets max/find read the E2 PSUM directly.
            m8p = gsb.tile([RT, 8], F32, tag=f"m8p{g}")
            nc.vector.max(m8p[:, :], ps_e2[:, :])
            i8p = gsb.tile([RT, 8], U32, tag=f"i8p{g}")
            nc.vector.max_index(out=i8p[:, :], in_max=m8p[:, :], in_values=ps_e2[:, :])
            kqf = gsb.tile([RT, 1], F32, tag=f"kqf{g}")
            nc.vector.tensor_copy(out=kqf[:, :], in_=i8p[:, 0:1])
            onehot = gsb.tile([RT, P], F32, tag=f"oh{g}")
            nc.vector.tensor_scalar(out=onehot[:, :], in0=iota93[:, :],
                                    scalar1=kqf[:, :], scalar2=None, op0=AL.is_equal)
            onehots.append(onehot)

        # ---- w/sb row broadcasts (only needed by dn2/sp, after the E2 work)
        ps_b = ps.tile([RT, P], F32, tag="EP")
        nc.tensor.matmul(ps_b[:, :], lhsT=onescol[:, :], rhs=wrow[:, :],
                         start=True, stop=True)
        w_bcast = gsb0.tile([RT, P], F32)
        nc.vector.tensor_copy(out=w_bcast[:, :], in_=ps_b[:, :])
        ps_b2 = ps.tile([RT, P], F32, tag="EP")
        nc.tensor.matmul(ps_b2[:, :], lhsT=onescol[:, :], rhs=sbrow[:, :],
                         start=True, stop=True)
        sb_bcast = gsb0.tile([RT, P], F32)
        nc.vector.tensor_copy(out=sb_bcast[:, :], in_=ps_b2[:, :])

        # ---- squared columns (scalar; only dep is ztl DMA)
        sqs = []
        for g in range(G):
            sq = gsb.tile([Z, W], F32R, tag=f"sq{g}")
            nc.scalar.activation(out=sq[:, :], in_=zTr[:, g * W:(g + 1) * W],
                                 func=AF.Square)
            sqs.append(sq)

        # ---- sp on pool (off the critical selection chain)
        sps = []
        for g in range(G):
            scr = gsb.tile([RT, P], F32, tag=f"pscr{g}")
            nc.gpsimd.tensor_tensor(out=scr[:, :], in0=onehots[g][:, :],
                                    in1=sb_bcast[:, :], op=AL.mult)
            sp = gsb.tile([RT, 1], F32, tag=f"sp{g}")
            nc.vector.tensor_reduce(out=sp[:, :], in_=scr[:, :], axis=AX.X, op=AL.add)
            sps.append(sp)

        # ---- prelim stage B: selected-proxy matrix per group
        mproxTs = []
        for g in range(G):
            ps_o = ps_oh.tile([P, RT], F32, tag="OH")
            nc.tensor.transpose(out=ps_o[:, :], in_=onehots[g][:, :],
                                identity=identity[:, :])
            ohT = gsb.tile([P, RT], F32, tag=f"ohT{g}")
            nc.scalar.copy(out=ohT[:, :], in_=ps_o[:, :])
            ps_pp = ps.tile([Z, RT], F32, tag="PP")
            nc.tensor.matmul(ps_pp[:, :], lhsT=m2prx[:, :], rhs=ohT[:, :],
                             start=True, stop=True)
            mproxT = gsb.tile([Z, RT], F32R, tag=f"mpx{g}")
            nc.scalar.copy(out=mproxT[:, :], in_=ps_pp[:, :])
            mproxTs.append(mproxT)

        # ---- mains: scores -> strip -> max/argmax  (+ index chain on pool,
        #      directly ahead of its gather in the same queue)
        strips, m8s, i8s, zps = [], [], [], []
        for g in range(G):
            c0 = g * W
            ps_S = ps_s.tile([RT, W], F32, tag="S")
            nc.tensor.matmul(ps_S[:, :], lhsT=mproxTs[g][:, :],
                             rhs=zTr[:, c0:c0 + W], start=True, stop=False)
            nc.tensor.matmul(ps_S[:, :], lhsT=ones128[:, :].bitcast(F32R),
                             rhs=sqs[g][:, :], start=False, stop=True)
            strip = gsb.tile([RT, W], F32, tag=f"st{g}")
            nc.vector.scalar_tensor_tensor(
                out=strip[:, :], in0=ps_S[:, :], scalar=1.0,
                in1=maskT[:, c0:c0 + W], op0=AL.mult, op1=AL.add)
            m8 = gsb.tile([RT, 8], F32, tag=f"m8{g}")
            nc.vector.max(m8[:, :], strip[:, :])
            i8 = gsb.tile([RT, 8], U32, tag=f"i8{g}")
            nc.vector.max_index(out=i8[:, :], in_max=m8[:, :], in_values=strip[:, :])
            zp = gsb.tile([RT, Z], F32, tag=f"zp{g}")
            nc.gpsimd.indirect_dma_start(
                out=zp[:, :], out_offset=None, in_=zrlp[:, :],
                in_offset=bass.IndirectOffsetOnAxis(ap=i8[:, 0:1], axis=0),
                element_offset=c0 * Z)
            m8s.append(m8)
            zps.append(zp)

        # ---- per-group epilogue: D_n / D_p / logsumexp (single act table)
        for g in range(G):
            zp = zps[g]
            nc.gpsimd.tensor_copy(out=outbuf[:, 2 * g:2 * g + 1], in_=m8s[g][:, 0:1])
            scr2 = gsb.tile([RT, Z], F32, tag=f"scr2{g}")
            zzjp = gsb.tile([RT, 1], F32, tag=f"zzjp{g}")
            nc.gpsimd.tensor_tensor(out=scr2[:, :], in0=zp[:, :], in1=zp[:, :],
                                    op=AL.mult)
            nc.vector.tensor_reduce(out=zzjp[:, :], in_=scr2[:, :], axis=AX.X,
                                    op=AL.add)
            szjp = gsb.tile([RT, 1], F32, tag=f"szjp{g}")
            nc.vector.tensor_reduce(out=szjp[:, :], in_=zp[:, :], axis=AX.X, op=AL.add)
            ps_zt = ps.tile([Z, RT], F32, tag="PP")
            nc.tensor.transpose(out=ps_zt[:, :], in_=zp[:, :], identity=identity[:, :])
            zpT = gsb.tile([Z, RT], F32, tag=f"zpT{g}")
            nc.vector.tensor_copy(out=zpT[:, :], in_=ps_zt[:, :])
            ps_dn = ps.tile([RT, P], F32, tag="EP")
            nc.tensor.matmul(ps_dn[:, :], lhsT=zpT[:, :], rhs=mprxT[:, :],
                             start=True, stop=True)
            zc = gsb.tile([RT, 1], F32, tag=f"zc{g}")
            nc.gpsimd.tensor_scalar(out=zc[:, :], in0=szjp[:, :], scalar1=EPS2,
                                    scalar2=ZEPS2, op0=AL.mult, op1=AL.add)
            nc.gpsimd.tensor_tensor(out=zc[:, :], in0=zc[:, :], in1=zzjp[:, :],
                                    op=AL.add)
            dn2 = gsb.tile([RT, P], F32, tag=f"dn2{g}")
            nc.vector.scalar_tensor_tensor(
                out=dn2[:, :], in0=ps_dn[:, :], scalar=zc[:, :], in1=w_bcast[:, :],
                op0=AL.add, op1=AL.add)
            scr3 = gsb.tile([RT, P], F32, tag=f"pscr{g}")
            dsel = gsb.tile([RT, 1], F32, tag=f"dsel{g}")
            nc.vector.tensor_tensor(out=scr3[:, :], in0=dn2[:, :], in1=onehots[g][:, :],
                                    op=AL.mult)
            nc.vector.tensor_reduce(out=dsel[:, :], in_=scr3[:, :], axis=AX.X,
                                    op=AL.add)
            # dn2 >= ~10 for real data (squared proxy-to-sample distances);
            # dn = sqrt(dn2) = exp(0.5*ln(dn2)); sume = sum exp(-dn)
            lnd = gsb.tile([RT, P], F32, tag=f"lnd{g}")
            nc.scalar.activation(out=lnd[:, :], in_=dn2[:, :], func=AF.Ln)
            dn = gsb.tile([RT, P], F32, tag=f"dn{g}")
            nc.scalar.activation(out=dn[:, :], in_=lnd[:, :], func=AF.Exp, scale=0.5)
            sume = gsb.tile([RT, 1], F32, tag=f"sume{g}")
            expd = gsb.tile([RT, P], F32, tag=f"lnd{g}")
            nc.scalar.activation(out=expd[:, :], in_=dn[:, :], func=AF.Exp,
                                 scale=-1.0, accum_out=sume[:, :])
            dp = gsb.tile([RT, 1], F32, tag=f"dp{g}")
            nc.vector.tensor_tensor(out=dp[:, :], in0=sps[g][:, :], in1=szjp[:, :],
                                    op=AL.subtract)
            nc.vector.scalar_tensor_tensor(
                out=dp[:, :], in0=dp[:, :], scalar=4.0 * EPS, in1=dsel[:, :],
                op0=AL.mult, op1=AL.add)
            nc.scalar.activation(out=dp[:, :], in_=dp[:, :], func=AF.Ln)
            nc.scalar.activation(out=dp[:, :], in_=dp[:, :], func=AF.Exp, scale=0.5)
            lse = gsb.tile([RT, 1], F32, tag=f"lse{g}")
            nc.scalar.activation(out=lse[:, :], in_=sume[:, :], func=AF.Ln)
            nc.vector.tensor_tensor(out=outbuf[:, 2 * g + 1:2 * g + 2],
                                    in0=dp[:, :], in1=lse[:, :], op=AL.add)

        nc.sync.dma_start(out=out[:, :], in_=outbuf[:, :])

    nc.finalize()
    return nc


def prep_inputs(z, y_idx, proxies, y_map):
    """Host-side sharding/layout prep (index math only; no float arithmetic
    on the input values)."""
    bf16 = ml_dtypes.bfloat16
    z = np.asarray(z, dtype=np.float32)
    y = np.asarray(y_idx, dtype=np.int32)
    y_map = np.asarray(y_map, dtype=np.int32)
    lut = np.zeros(int(y_map.max()) + 1, dtype=np.int32)
    lut[y_map] = np.arange(len(y_map), dtype=np.int32)
    yrel = lut[y]

    anchors = np.arange(0, B - 3, 3, dtype=np.int64)
    acls = yrel[anchors]
    ncls = len(y_map)

    na = np.bincount(acls, minlength=ncls)
    ncc = np.bincount(yrel, minlength=ncls)
    used = np.where(na > 0)[0]
    order = used[np.argsort(-na[used], kind="stable")]

    def pack(colcap):
        bins = []
        for ci in order:
            for b in bins:
                if b[0] + na[ci] <= RT and b[1] + ncc[ci] <= colcap:
                    b[0] += na[ci]; b[1] += ncc[ci]; b[2].append(int(ci))
                    break
            else:
                bins.append([int(na[ci]), int(ncc[ci]), [int(ci)]])
        return bins

    bins = None
    for cap in (384, 416, 448, 480, 512):
        cand = pack(cap)
        if len(cand) <= 3 * NCORE:
            bins = cand
            break
    if bins is None:
        bins = pack(512)
    G = (len(bins) + NCORE - 1) // NCORE
    W = max(b[1] for b in bins)
    W = min(512, (W + 7) // 8 * 8)

    cls_arow = [np.where(acls == c)[0] for c in range(ncls)]
    cls_cols = [np.where(yrel == c)[0] for c in range(ncls)]

    zT = np.ascontiguousarray(z.T)
    iota93 = np.broadcast_to(np.arange(P, dtype=np.float32), (RT, P)).copy()
    prxf = np.asarray(proxies, dtype=np.float32)

    in_maps = []
    mapping = []  # (core, g, n_anchors)
    for c in range(NCORE):
        ztl = np.zeros((Z, G * W), dtype=np.float32)
        zatl = np.zeros((Z, G * RT), dtype=np.float32)
        maskbt = np.full((RT, G * W), bf16(BIGNEG), dtype=bf16)
        zrlp = np.zeros((G * W, Z), dtype=np.float32)
        for g in range(G):
            bi = c + NCORE * g
            if bi >= len(bins):
                continue
            classes = bins[bi][2]
            arow = np.concatenate([cls_arow[cc] for cc in classes])
            cols = np.concatenate([cls_cols[cc] for cc in classes])
            nr, nw = len(arow), len(cols)
            a_idx = anchors[arow]
            zatl[:, g * RT:g * RT + nr] = zT[:, a_idx]
            ztl[:, g * W:g * W + nw] = zT[:, cols]
            zrlp[g * W:g * W + nw, :] = z[cols, :]
            ok = (yrel[cols][None, :] == acls[arow][:, None]) & \
                 (cols[None, :] >= a_idx[:, None])
            blk = np.full((nr, nw), bf16(BIGNEG), dtype=bf16)
            blk[ok] = bf16(0.0)
            maskbt[:nr, g * W:g * W + nw] = blk
            mapping.append((c, g, nr))
        in_maps.append({
            "ztl": ztl, "zatl": zatl, "maskb": maskbt, "zrlp": zrlp,
            "prx": prxf, "iota93": iota93,
        })
    return G, W, in_maps, mapping, len(anchors)


def combine(results, mapping, A):
    total = 0.0
    for c, g, nr in mapping:
        total += results[c]["out"][:nr, 2 * g + 1].astype(np.float64).sum()
    return np.float32(total / A)


def kernel(z, y_idx, proxies, y_map, _trace=False):
    G, W, in_maps, mapping, A = prep_inputs(z, y_idx, proxies, y_map)
    key = (G, W)
    if key not in _CACHE:
        _CACHE[key] = build_program(G, W)
    nc = _CACHE[key]
    res = run_bass_kernel_spmd(nc, in_maps, core_ids=list(range(NCORE)),
                               trace=_trace)
    outv = combine(res.results, mapping, A)
    if _trace:
        return outv, res
    return outv


if __name__ == "__main__":
    import jax
    with jax.default_device(jax.devices("cpu")[0]):
        import reference
        inputs = {k: np.asarray(v) for k, v in reference.setup_inputs().items()}
        expected = np.asarray(jax.jit(reference.reference, backend="cpu")(**inputs))
    actual = kernel(**inputs)
    rel = abs(float(actual) - float(expected)) / max(abs(float(expected)), 1e-12)
    print(f"expected {expected}, actual {actual}, rel err {rel:.3e}")


# revision 54
# speedup vs baseline: 1.1937x; 1.1937x over previous
"""DynamicProxyNCA loss on 8 TRN2 NeuronCores (Bass/Tile, SPMD).

Class-block decomposition: the hardest-positive argmax for an anchor only
ranges over same-class columns in its suffix (~132 of 8192), so scores are
computed only within class blocks instead of over the full batch.

  - Host (index prep only): permute columns by class; FFD-pack classes into
    bins of <=128 anchors x <=W columns; bin b -> core b%8, group b//8.
    Suffix+class+padding constraints become one additive bf16 mask per bin.
  - Device, per group g: nearest-proxy selection for the group's anchors
    (E2 = 2<z_a,p> - w, max8/find_index), then ONE f32r matmul pair builds
    scores S = -2<p_sel, z_j> + zz_j directly in PSUM (zz via an all-ones
    lhsT over squared columns), + mask add -> strip -> max8/find_index.
  - Hardest positive gathered by indirect DMA from the permuted row table
    (f32, exact); D_p / D_n / logsumexp recomputed in f32 as in reference.
  - All sqrt as exp(0.5*ln(x)): the whole program runs on ONE scalar
    activation table (natural_log_exp covers Square/Copy/Exp/Ln).
  - ||prx_n||^2 == 1 by construction, so bb is the constant 1.0.
  - Host sums per-anchor losses (pure gather/reduce of per-core outputs).
"""
import sys

sys.path.insert(0, "/opt/trn_rl_repo")

import numpy as np
import ml_dtypes

import concourse.bass as bass
import concourse.tile as tile
from concourse import bacc, mybir
from concourse.bass_utils import run_bass_kernel_spmd
from concourse.masks import make_identity

# This kernel only uses Square/Copy/Exp/Ln on the scalar engine, all of which
# live together in the natural_log_exp_and_others activation table. The
# default table-selection pass resolves each function to the FIRST table
# containing it, which thrashes between exp_and_others and natural_log
# (1.5us reload each). Restrict resolution to the one covering table while
# preserving table ids (dict order); runtime behavior is identical since that
# table genuinely contains every function used.
from concourse.hw_specs import get_activation_tables as _real_gat

_COVER_SET = "natural_log_exp_and_others"


def _gat_single(arch):
    tabs = _real_gat(arch)
    return {n: (f if n == _COVER_SET else set()) for n, f in tabs.items()}


bacc.get_activation_tables = _gat_single

F32 = mybir.dt.float32
F32R = mybir.dt.float32r
BF16 = mybir.dt.bfloat16
U32 = mybir.dt.uint32

B, Z = 8192, 128
P = 93
EPS = 1e-6
RT = 128                # max anchors per group
NCORE = 8
BIGNEG = -9984.0        # exact in bf16; far below any real score (>= -1)
EPS2 = 2.0 * EPS
ZEPS2 = Z * EPS * EPS

_CACHE = {}


def build_program(G, W):
    nc = bacc.Bacc(None, target_bir_lowering=False, debug=False)

    ztl = nc.dram_tensor("ztl", [Z, G * W], F32R, kind="ExternalInput")
    zatl = nc.dram_tensor("zatl", [Z, G * RT], F32, kind="ExternalInput")
    maskb = nc.dram_tensor("maskb", [RT, G * W], BF16, kind="ExternalInput")
    zrlp = nc.dram_tensor("zrlp", [G * W, Z], F32, kind="ExternalInput")
    prx_in = nc.dram_tensor("prx", [P, Z], F32, kind="ExternalInput")
    iota93_in = nc.dram_tensor("iota93", [RT, P], F32, kind="ExternalInput")
    out = nc.dram_tensor("out", [RT, 2 * G], F32, kind="ExternalOutput")

    AL = mybir.AluOpType
    AF = mybir.ActivationFunctionType
    AX = mybir.AxisListType

    from contextlib import ExitStack

    with tile.TileContext(nc) as tc, ExitStack() as ctx:
        singles = ctx.enter_context(tc.tile_pool(name="singles", bufs=1))

        prx = singles.tile([P, Z], F32)
        nc.scalar.dma_start(out=prx[:, :], in_=prx_in[:, :])
        zat = singles.tile([Z, G * RT], F32)
        nc.sync.dma_start(out=zat[:, :], in_=zatl[:, :])
        iota93 = singles.tile([RT, P], F32)
        nc.scalar.dma_start(out=iota93[:, :], in_=iota93_in[:, :])
        zTr = singles.tile([Z, G * W], F32R)
        nc.sync.dma_start(out=zTr[:, :], in_=ztl[:, :])
        maskT = singles.tile([RT, G * W], BF16)
        nc.scalar.dma_start(out=maskT[:, :], in_=maskb[:, :])

        identity = singles.tile([128, 128], F32)
        make_identity(nc, identity[:, :])
        onescol = singles.tile([1, RT], F32)
        nc.vector.memset(onescol[:, :], 1.0)
        ones128 = singles.tile([Z, RT], F32)
        nc.vector.memset(ones128[:, :], 1.0)
        outbuf = singles.tile([RT, 2 * G], F32)

        # ---- proxy preprocessing (sqrt-free: rn = exp(-0.5*ln(ss)))
        with tc.tile_pool(name="setup_sb", bufs=1) as stp, \
             tc.tile_pool(name="setup_ps", bufs=1, space="PSUM") as stps:
            # PE p-state warm-up: ~3.5us of back-to-back dummy matmuls in the
            # otherwise-idle window while input DMAs land, ramping the tensor
            # engine to full clock before the real matmuls (which otherwise
            # sustain mid-p-state timing). No data deps beyond the memset.
            warm = stps.tile([RT, RT], F32, tag="WRM")
            for _ in range(10):
                nc.tensor.matmul(warm[:, :], lhsT=ones128[:, :],
                                 rhs=ones128[:, :], start=True, stop=True)
            scratch = stp.tile([P, Z], F32)
            ss = stp.tile([P, 1], F32)
            nc.scalar.activation(out=scratch[:, :], in_=prx[:, :], func=AF.Square,
                                 accum_out=ss[:, :])
            lnss = stp.tile([P, 1], F32)
            nc.scalar.activation(out=lnss[:, :], in_=ss[:, :], func=AF.Ln)
            rn = stp.tile([P, 1], F32)
            nc.scalar.activation(out=rn[:, :], in_=lnss[:, :], func=AF.Exp,
                                 scale=-0.5)
            prx_n = stp.tile([P, Z], F32)
            nc.vector.tensor_scalar_mul(out=prx_n[:, :], in0=prx[:, :], scalar1=rn[:, :])
            m2prx = singles.tile([P, Z], F32)
            nc.vector.tensor_scalar_mul(out=m2prx[:, :], in0=prx_n[:, :], scalar1=-2.0)
            sb0 = stp.tile([P, 1], F32)
            nc.vector.tensor_reduce(out=sb0[:, :], in_=prx_n[:, :], axis=AX.X, op=AL.add)

            ps_t = stps.tile([Z, P], F32, tag="pst")
            nc.tensor.transpose(out=ps_t[:, :], in_=prx_n[:, :], identity=identity[:P, :P])
            prxT2 = singles.tile([Z, P], F32)
            nc.scalar.mul(out=prxT2[:, :], in_=ps_t[:, :], mul=2.0)
            mprxT = singles.tile([Z, P], F32)
            nc.scalar.mul(out=mprxT[:, :], in_=ps_t[:, :], mul=-2.0)

            ps_r2 = stps.tile([1, P], F32, tag="psr")
            nc.tensor.transpose(out=ps_r2[:, :], in_=sb0[:, :], identity=identity[:P, :P])
            sbrow = singles.tile([1, P], F32)
            nc.vector.tensor_copy(out=sbrow[:, :], in_=ps_r2[:, :])
            # bb == ||prx_n||^2 == 1 by construction: w = 1 - 2*eps*sb
            wrow = singles.tile([1, P], F32)
            nc.vector.tensor_scalar(out=wrow[:, :], in0=sbrow[:, :], scalar1=-EPS2,
                                    scalar2=1.0, op0=AL.mult, op1=AL.add)

        gsb0 = ctx.enter_context(tc.tile_pool(name="gsb0", bufs=1))

        gsb = ctx.enter_context(tc.tile_pool(name="gsb", bufs=1))
        ps = ctx.enter_context(tc.tile_pool(name="ps", bufs=2, space="PSUM"))
        ps_s = ctx.enter_context(tc.tile_pool(name="ps_s", bufs=2, space="PSUM"))
        ps_oh = ctx.enter_context(tc.tile_pool(name="ps_oh", bufs=1, space="PSUM"))

        # ---- prelim stage A: nearest proxy per anchor (argmax of E2)
        onehots = []
        for g in range(G):
            a0 = g * RT
            ps_e2 = ps.tile([RT, P], F32, tag="EP")
            nc.tensor.matmul(ps_e2[:, :], lhsT=zat[:, a0:a0 + RT], rhs=prxT2[:, :],
                             start=True, stop=True)
            # argmin tie-term -2eps*sb_k (~2e-5) is 50x below f32r matmul
            # noise; dropping it lets max/find read the E2 PSUM directly.
            m8p = gsb.tile([RT, 8], F32, tag=f"m8p{g}")
            nc.vector.max(m8p[:, :], ps_e2[:, :])
            i8p = gsb.tile([RT, 8], U32, tag=f"i8p{g}")
            nc.vector.max_index(out=i8p[:, :], in_max=m8p[:, :], in_values=ps_e2[:, :])
            kqf = gsb.tile([RT, 1], F32, tag=f"kqf{g}")
            nc.vector.tensor_copy(out=kqf[:, :], in_=i8p[:, 0:1])
            onehot = gsb.tile([RT, P], F32, tag=f"oh{g}")
            nc.vector.tensor_scalar(out=onehot[:, :], in0=iota93[:, :],
                                    scalar1=kqf[:, :], scalar2=None, op0=AL.is_equal)
            onehots.append(onehot)

        # ---- w/sb row broadcasts (only needed by dn2/sp, after the E2 work)
        ps_b = ps.tile([RT, P], F32, tag="EP")
        nc.tensor.matmul(ps_b[:, :], lhsT=onescol[:, :], rhs=wrow[:, :],
                         start=True, stop=True)
        w_bcast = gsb0.tile([RT, P], F32)
        nc.vector.tensor_copy(out=w_bcast[:, :], in_=ps_b[:, :])
        ps_b2 = ps.tile([RT, P], F32, tag="EP")
        nc.tensor.matmul(ps_b2[:, :], lhsT=onescol[:, :], rhs=sbrow[:, :],
                         start=True, stop=True)
        sb_bcast = gsb0.tile([RT, P], F32)
        nc.vector.tensor_copy(out=sb_bcast[:, :], in_=ps_b2[:, :])

        # ---- squared columns (scalar; only dep is ztl DMA)
        sqs = []
        for g in range(G):
            sq = gsb.tile([Z, W], F32R, tag=f"sq{g}")
            nc.scalar.activation(out=sq[:, :], in_=zTr[:, g * W:(g + 1) * W],
                                 func=AF.Square)
            sqs.append(sq)

        # ---- sp on pool (off the critical selection chain)
        sps = []
        for g in range(G):
            scr = gsb.tile([RT, P], F32, tag=f"pscr{g}")
            nc.gpsimd.tensor_tensor(out=scr[:, :], in0=onehots[g][:, :],
                                    in1=sb_bcast[:, :], op=AL.mult)
            sp = gsb.tile([RT, 1], F32, tag=f"sp{g}")
            nc.vector.tensor_reduce(out=sp[:, :], in_=scr[:, :], axis=AX.X, op=AL.add)
            sps.append(sp)

        # ---- prelim stage B: selected-proxy matrix per group
        mproxTs = []
        for g in range(G):
            ps_o = ps_oh.tile([P, RT], F32, tag="OH")
            nc.tensor.transpose(out=ps_o[:, :], in_=onehots[g][:, :],
                                identity=identity[:, :])
            ohT = gsb.tile([P, RT], F32, tag=f"ohT{g}")
            nc.scalar.copy(out=ohT[:, :], in_=ps_o[:, :])
            ps_pp = ps.tile([Z, RT], F32, tag="PP")
            nc.tensor.matmul(ps_pp[:, :], lhsT=m2prx[:, :], rhs=ohT[:, :],
                             start=True, stop=True)
            mproxT = gsb.tile([Z, RT], F32R, tag=f"mpx{g}")
            nc.scalar.copy(out=mproxT[:, :], in_=ps_pp[:, :])
            mproxTs.append(mproxT)

        # ---- mains: scores -> strip -> max/argmax  (+ index chain on pool,
        #      directly ahead of its gather in the same queue)
        strips, m8s, i8s, zps = [], [], [], []
        for g in range(G):
            c0 = g * W
            ps_S = ps_s.tile([RT, W], F32, tag="S")
            nc.tensor.matmul(ps_S[:, :], lhsT=mproxTs[g][:, :],
                             rhs=zTr[:, c0:c0 + W], start=True, stop=False)
            nc.tensor.matmul(ps_S[:, :], lhsT=ones128[:, :].bitcast(F32R),
                             rhs=sqs[g][:, :], start=False, stop=True)
            strip = gsb.tile([RT, W], F32, tag=f"st{g}")
            nc.vector.scalar_tensor_tensor(
                out=strip[:, :], in0=ps_S[:, :], scalar=1.0,
                in1=maskT[:, c0:c0 + W], op0=AL.mult, op1=AL.add)
            m8 = gsb.tile([RT, 8], F32, tag=f"m8{g}")
            nc.vector.max(m8[:, :], strip[:, :])
            i8 = gsb.tile([RT, 8], U32, tag=f"i8{g}")
            nc.vector.max_index(out=i8[:, :], in_max=m8[:, :], in_values=strip[:, :])
            zp = gsb.tile([RT, Z], F32, tag=f"zp{g}")
            nc.gpsimd.indirect_dma_start(
                out=zp[:, :], out_offset=None, in_=zrlp[:, :],
                in_offset=bass.IndirectOffsetOnAxis(ap=i8[:, 0:1], axis=0),
                element_offset=c0 * Z)
            m8s.append(m8)
            zps.append(zp)

        # ---- per-group epilogue: D_n / D_p / logsumexp (single act table)
        for g in range(G):
            zp = zps[g]
            nc.gpsimd.tensor_copy(out=outbuf[:, 2 * g:2 * g + 1], in_=m8s[g][:, 0:1])
            scr2 = gsb.tile([RT, Z], F32, tag=f"scr2{g}")
            zzjp = gsb.tile([RT, 1], F32, tag=f"zzjp{g}")
            nc.gpsimd.tensor_tensor(out=scr2[:, :], in0=zp[:, :], in1=zp[:, :],
                                    op=AL.mult)
            nc.vector.tensor_reduce(out=zzjp[:, :], in_=scr2[:, :], axis=AX.X,
                                    op=AL.add)
            szjp = gsb.tile([RT, 1], F32, tag=f"szjp{g}")
            nc.vector.tensor_reduce(out=szjp[:, :], in_=zp[:, :], axis=AX.X, op=AL.add)
            ps_zt = ps.tile([Z, RT], F32, tag="PP")
            nc.tensor.transpose(out=ps_zt[:, :], in_=zp[:, :], identity=identity[:, :])
            zpT = gsb.tile([Z, RT], F32, tag=f"zpT{g}")
            nc.vector.tensor_copy(out=zpT[:, :], in_=ps_zt[:, :])
            ps_dn = ps.tile([RT, P], F32, tag="EP")
            nc.tensor.matmul(ps_dn[:, :], lhsT=zpT[:, :], rhs=mprxT[:, :],
                             start=True, stop=True)
            zc = gsb.tile([RT, 1], F32, tag=f"zc{g}")
            nc.gpsimd.tensor_scalar(out=zc[:, :], in0=szjp[:, :], scalar1=EPS2,
                                    scalar2=ZEPS2, op0=AL.mult, op1=AL.add)
            nc.gpsimd.tensor_tensor(out=zc[:, :], in0=zc[:, :], in1=zzjp[:, :],
                                    op=AL.add)
            dn2 = gsb.tile([RT, P], F32, tag=f"dn2{g}")
            nc.vector.scalar_tensor_tensor(
                out=dn2[:, :], in0=ps_dn[:, :], scalar=zc[:, :], in1=w_bcast[:, :],
                op0=AL.add, op1=AL.add)
            scr3 = gsb.tile([RT, P], F32, tag=f"pscr{g}")
            dsel = gsb.tile([RT, 1], F32, tag=f"dsel{g}")
            nc.vector.tensor_tensor(out=scr3[:, :], in0=dn2[:, :], in1=onehots[g][:, :],
                                    op=AL.mult)
            nc.vector.tensor_reduce(out=dsel[:, :], in_=scr3[:, :], axis=AX.X,
                                    op=AL.add)
            # dn2 >= ~10 for real data (squared proxy-to-sample distances);
            # dn = sqrt(dn2) = exp(0.5*ln(dn2)); sume = sum exp(-dn)
            lnd = gsb.tile([RT, P], F32, tag=f"lnd{g}")
            nc.scalar.activation(out=lnd[:, :], in_=dn2[:, :], func=AF.Ln)
            dn = gsb.tile([RT, P], F32, tag=f"dn{g}")
            nc.scalar.activation(out=dn[:, :], in_=lnd[:, :], func=AF.Exp, scale=0.5)
            sume = gsb.tile([RT, 1], F32, tag=f"sume{g}")
            expd = gsb.tile([RT, P], F32, tag=f"lnd{g}")
            nc.scalar.activation(out=expd[:, :], in_=dn[:, :], func=AF.Exp,
                                 scale=-1.0, accum_out=sume[:, :])
            dp = gsb.tile([RT, 1], F32, tag=f"dp{g}")
            nc.vector.tensor_tensor(out=dp[:, :], in0=sps[g][:, :], in1=szjp[:, :],
                                    op=AL.subtract)
            nc.vector.scalar_tensor_tensor(
                out=dp[:, :], in0=dp[:, :], scalar=4.0 * EPS, in1=dsel[:, :],
                op0=AL.mult, op1=AL.add)
            nc.scalar.activation(out=dp[:, :], in_=dp[:, :], func=AF.Ln)
            nc.scalar.activation(out=dp[:, :], in_=dp[:, :], func=AF.Exp, scale=0.5)
            lse = gsb.tile([RT, 1], F32, tag=f"lse{g}")
            nc.scalar.activation(out=lse[:, :], in_=sume[:, :], func=AF.Ln)
            nc.vector.tensor_tensor(out=outbuf[:, 2 * g + 1:2 * g + 2],
                                    in0=dp[:, :], in1=lse[:, :], op=AL.add)

        nc.sync.dma_start(out=out[:, :], in_=outbuf[:, :])

    nc.finalize()
    return nc


def prep_inputs(z, y_idx, proxies, y_map):
    """Host-side sharding/layout prep (index math only; no float arithmetic
    on the input values)."""
    bf16 = ml_dtypes.bfloat16
    z = np.asarray(z, dtype=np.float32)
    y = np.asarray(y_idx, dtype=np.int32)
    y_map = np.asarray(y_map, dtype=np.int32)
    lut = np.zeros(int(y_map.max()) + 1, dtype=np.int32)
    lut[y_map] = np.arange(len(y_map), dtype=np.int32)
    yrel = lut[y]

    anchors = np.arange(0, B - 3, 3, dtype=np.int64)
    acls = yrel[anchors]
    ncls = len(y_map)

    na = np.bincount(acls, minlength=ncls)
    ncc = np.bincount(yrel, minlength=ncls)
    used = np.where(na > 0)[0]
    order = used[np.argsort(-na[used], kind="stable")]

    def pack(colcap):
        bins = []
        for ci in order:
            for b in bins:
                if b[0] + na[ci] <= RT and b[1] + ncc[ci] <= colcap:
                    b[0] += na[ci]; b[1] += ncc[ci]; b[2].append(int(ci))
                    break
            else:
                bins.append([int(na[ci]), int(ncc[ci]), [int(ci)]])
        return bins

    bins = None
    for cap in (384, 416, 448, 480, 512):
        cand = pack(cap)
        if len(cand) <= 3 * NCORE:
            bins = cand
            break
    if bins is None:
        bins = pack(512)
    G = (len(bins) + NCORE - 1) // NCORE
    W = max(b[1] for b in bins)
    W = min(512, (W + 7) // 8 * 8)

    cls_arow = [np.where(acls == c)[0] for c in range(ncls)]
    cls_cols = [np.where(yrel == c)[0] for c in range(ncls)]

    zT = np.ascontiguousarray(z.T)
    iota93 = np.broadcast_to(np.arange(P, dtype=np.float32), (RT, P)).copy()
    prxf = np.asarray(proxies, dtype=np.float32)

    in_maps = []
    mapping = []  # (core, g, n_anchors)
    for c in range(NCORE):
        ztl = np.zeros((Z, G * W), dtype=np.float32)
        zatl = np.zeros((Z, G * RT), dtype=np.float32)
        maskbt = np.full((RT, G * W), bf16(BIGNEG), dtype=bf16)
        zrlp = np.zeros((G * W, Z), dtype=np.float32)
        for g in range(G):
            bi = c + NCORE * g
            if bi >= len(bins):
                continue
            classes = bins[bi][2]
            arow = np.concatenate([cls_arow[cc] for cc in classes])
            cols = np.concatenate([cls_cols[cc] for cc in classes])
            nr, nw = len(arow), len(cols)
            a_idx = anchors[arow]
            zatl[:, g * RT:g * RT + nr] = zT[:, a_idx]
            ztl[:, g * W:g * W + nw] = zT[:, cols]
            zrlp[g * W:g * W + nw, :] = z[cols, :]
            ok = (yrel[cols][None, :] == acls[arow][:, None]) & \
                 (cols[None, :] >= a_idx[:, None])
            blk = np.full((nr, nw), bf16(BIGNEG), dtype=bf16)
            blk[ok] = bf16(0.0)
            maskbt[:nr, g * W:g * W + nw] = blk
            mapping.append((c, g, nr))
        in_maps.append({
            "ztl": ztl, "zatl": zatl, "maskb": maskbt, "zrlp": zrlp,
            "prx": prxf, "iota93": iota93,
        })
    return G, W, in_maps, mapping, len(anchors)


def combine(results, mapping, A):
    total = 0.0
    for c, g, nr in mapping:
        total += results[c]["out"][:nr, 2 * g + 1].astype(np.float64).sum()
    return np.float32(total / A)


def kernel(z, y_idx, proxies, y_map, _trace=False):
    G, W, in_maps, mapping, A = prep_inputs(z, y_idx, proxies, y_map)
    key = (G, W)
    if key not in _CACHE:
        _CACHE[key] = build_program(G, W)
    nc = _CACHE[key]
    res = run_bass_kernel_spmd(nc, in_maps, core_ids=list(range(NCORE)),
                               trace=_trace)
    outv = combine(res.results, mapping, A)
    if _trace:
        return outv, res
    return outv


if __name__ == "__main__":
    import jax
    with jax.default_device(jax.devices("cpu")[0]):
        import reference
        inputs = {k: np.asarray(v) for k, v in reference.setup_inputs().items()}
        expected = np.asarray(jax.jit(reference.reference, backend="cpu")(**inputs))
    actual = kernel(**inputs)
    rel = abs(float(actual) - float(expected)) / max(abs(float(expected)), 1e-12)
    print(f"expected {expected}, actual {actual}, rel err {rel:.3e}")
